# revision 16
# baseline (speedup 1.0000x reference)
"""Trainium2 Bass kernel for the MixEHR SCVB0_un step (nn_MixEHR_5428838662489).

Math (see reference):
    a     = alpha + exp_m[batch_indices]                  [B, K]
    denom = beta.sum(0) + exp_n.sum(0)                    [K]
    b     = (beta + exp_n) / denom                        [V, K]
    Z     = a @ b.T                                       [B, V]
    W     = BOW / (Z + 1e-6)                              [B, V]
    out   = (1-rho) * exp_n + rho*scale * b * (W.T @ a)   [V, K]

Device strategy: shard the vocabulary V across the 8 cores (no collectives
needed — every core holds all B=512 documents and produces a complete
[V/8, K] output shard).  All matmul factors are prefolded on the host:

    s   = beta + exp_n                 (f32, natural + f16 transposed)
    aT1 = (a / denom).T                (f16)   Z = aT1.T @ sT
    a2  = a * (rho*scale/denom)        (f16)   P = W.T @ a2
    e   = (1-rho) * exp_n              (f32)
    out = e + s * P

Per core the vocab shard is padded 12500 -> 12800 = 25 blocks of 512.
Per block: 4 matmuls build Z [128d, 4*512v] in PSUM; reciprocal via
ACT exp(-ln(Z+eps)) (or DVE reciprocal_approx_fast on some units);
W = BOW * R on DVE (f16); 16 matmuls accumulate P [128v, 4*50k];
DVE combines out = e + s*P.
"""

import numpy as np

import concourse.bass as bass
import concourse.mybir as mybir
import concourse.tile as tile
from concourse import bacc
from concourse.bass_utils import run_bass_kernel_spmd

B = 512          # documents (batch)
V = 100000       # vocabulary
K = 50           # topics
NCORES = 8
VCORE = 12500    # true vocab per core
VPAD = 12800     # padded vocab per core
NBLK = 25        # blocks of 512 vocab per core
MINI = 1e-6

F16 = mybir.dt.float16
F32 = mybir.dt.float32

# Which (block, half) units use the DVE reciprocal path instead of the ACT
# exp(-ln(.)) path.  unit index = blk*2 + half, 50 units total.
DVE_UNIT_MOD = 10     # units with (idx % DVE_UNIT_MOD) in DVE_UNIT_SET go to DVE
DVE_UNIT_SET = ()     # v1: all units on the ACT path

_CACHE = {}
_last_results = None  # test harness reads timing info from here


def _build_nc():
    nc = bacc.Bacc("TRN2", target_bir_lowering=False)
    # Register the eps bias constant as an untracked const AP (written in the
    # preamble, before the TileContext) so activations using it need no
    # cross-engine sync wait — the ACT instruction has one wait slot.
    _eps = nc.alloc_sbuf_tensor("const-eps", [128, 1], F32)
    nc.gpsimd.memset(_eps.ap(), float(MINI))
    nc.const_aps.aps[(F32, float(MINI))] = _eps.ap()
    nc.all_engine_barrier()
    bow = nc.declare_dram_parameter("bow", [NBLK, 128, 2048], F16, isOutput=False)
    sT = nc.declare_dram_parameter("sT", [K, VPAD], F16, isOutput=False)
    s_p = nc.declare_dram_parameter("s", [NBLK, 128, 200], F32, isOutput=False)
    e_p = nc.declare_dram_parameter("e", [NBLK, 128, 200], F32, isOutput=False)
    aT1 = nc.declare_dram_parameter("aT1", [K, B], F16, isOutput=False)
    a2 = nc.declare_dram_parameter("a2", [128, 200], F16, isOutput=False)
    out = nc.declare_dram_parameter("out", [NBLK, 128, 200], F32, isOutput=True)

    with tile.TileContext(nc) as tc:
        with (
            tc.tile_pool(name="consts", bufs=1) as consts,
            tc.tile_pool(name="bowp", bufs=3) as bowp,
            tc.tile_pool(name="zp", bufs=6, space="PSUM") as zpool,
            tc.tile_pool(name="pp", bufs=2, space="PSUM") as ppool,
            tc.tile_pool(name="tp", bufs=4) as tpool,
            tc.tile_pool(name="rp", bufs=2) as rpool,
            tc.tile_pool(name="wp", bufs=2) as wpool,
            tc.tile_pool(name="sep", bufs=4) as sepool,
            tc.tile_pool(name="op", bufs=3) as opool,
        ):
            sT_t = consts.tile([K, VPAD], F16)
            nc.sync.dma_start(out=sT_t, in_=sT[:])
            aT1_t = consts.tile([K, B], F16)
            nc.sync.dma_start(out=aT1_t, in_=aT1[:])
            a2_t = consts.tile([128, 200], F16)
            nc.sync.dma_start(out=a2_t, in_=a2[:])


            for blk in range(NBLK):
                bow_t = bowp.tile([128, 2048], F16, tag="bow")
                nc.sync.dma_start(out=bow_t, in_=bow[blk])
                s_t = sepool.tile([128, 200], F32, tag="s")
                nc.sync.dma_start(out=s_t, in_=s_p[blk])
                e_t = sepool.tile([128, 200], F32, tag="e")
                nc.sync.dma_start(out=e_t, in_=e_p[blk])

                # ---- Z = aT1.T @ sT per doc-chunk: one [128d, 512v] tile each ----
                zp_c = []
                for c in range(4):
                    z_t = zpool.tile([128, 512], F32, tag="z")
                    zp_c.append(z_t)
                    nc.tensor.matmul(
                        z_t,
                        lhsT=aT1_t[:, c * 128 : (c + 1) * 128],
                        rhs=sT_t[:, blk * 512 : (blk + 1) * 512],
                        start=True,
                        stop=True,
                    )

                # ---- R = 1/(Z+eps) -> f16, then W = BOW * R (f16) ----
                r_t = rpool.tile([128, 2048], F16, tag="r")
                if (blk % DVE_UNIT_MOD) in DVE_UNIT_SET:
                    for c in range(4):
                        rf_t = tpool.tile([128, 512], F32, tag="t")
                        nc.vector.reciprocal_approx_fast(out=rf_t, in_=zp_c[c])
                        nc.vector.tensor_copy(r_t[:, c * 512 : (c + 1) * 512], rf_t)
                else:
                    t_t = tpool.tile([128, 2048], F32, tag="t")
                    for c in range(4):
                        nc.scalar.activation(
                            t_t[:, c * 512 : (c + 1) * 512],
                            zp_c[c],
                            mybir.ActivationFunctionType.Ln,
                            bias=float(MINI),
                            scale=1.0,
                        )
                    nc.scalar.activation(
                        r_t,
                        t_t,
                        mybir.ActivationFunctionType.Exp,
                        bias=0.0,
                        scale=-1.0,
                    )

                w_t = wpool.tile([128, 2048], F16, tag="w")
                nc.vector.tensor_mul(w_t, bow_t, r_t)

                # ---- P[v,k] = sum_d W[d,v] * a2[d,k], accumulated over 4 d-chunks
                p_t = ppool.tile([128, 200], F32, tag="p")
                for vs in range(4):
                    for c in range(4):
                        nc.tensor.matmul(
                            p_t[:, vs * 50 : vs * 50 + 50],
                            lhsT=w_t[:, c * 512 + vs * 128 : c * 512 + vs * 128 + 128],
                            rhs=a2_t[:, c * 50 : (c + 1) * 50],
                            start=(c == 0),
                            stop=(c == 3),
                        )

                # ---- out = e + s * P ----
                u_t = opool.tile([128, 200], F32, tag="u")
                nc.vector.tensor_mul(u_t, s_t, p_t)
                o_t = opool.tile([128, 200], F32, tag="o")
                nc.vector.tensor_add(o_t, u_t, e_t)
                nc.sync.dma_start(out=out[blk], in_=o_t)

    nc.compile()
    return nc


def _get_nc():
    if "nc" not in _CACHE:
        _CACHE["nc"] = _build_nc()
    return _CACHE["nc"]


def kernel(
    batch_BOW,
    alpha,
    beta,
    exp_m,
    exp_n,
    batch_indices,
    iter_n,
    batch_C,
    C_m,
):
    global _last_results
    BOW = np.asarray(batch_BOW, dtype=np.float32)
    alpha = np.asarray(alpha, dtype=np.float32)
    beta = np.asarray(beta, dtype=np.float32)
    exp_m = np.asarray(exp_m, dtype=np.float32)
    exp_n = np.asarray(exp_n, dtype=np.float32)
    bidx = np.asarray(batch_indices)

    rho = 1.0 / float(int(iter_n) + 5) ** 0.9
    scale = float(C_m) / float(batch_C)

    # ---- host prefolding (O(V*K) / O(B*K) prep) ----
    denom = (
        beta.sum(axis=0, dtype=np.float64) + exp_n.sum(axis=0, dtype=np.float64)
    ).astype(np.float32)
    a = alpha[None, :] + exp_m[bidx]                       # [B, K]
    aT1 = np.ascontiguousarray((a / denom[None, :]).T.astype(np.float16))
    a2 = (a * (rho * scale / denom)[None, :]).astype(np.float16)
    a2_pack = np.ascontiguousarray(
        a2.reshape(4, 128, K).transpose(1, 0, 2).reshape(128, 200)
    )
    s = beta + exp_n                                       # [V, K] f32

    VP = VPAD * NCORES
    sT_pad = np.ones((K, VP), dtype=np.float16)
    sT_pad[:, :V] = s.T.astype(np.float16)
    s_pad = np.zeros((VP, K), dtype=np.float32)
    s_pad[:V] = s
    e_pad = np.zeros((VP, K), dtype=np.float32)
    e_pad[:V] = (1.0 - rho) * exp_n
    bow_pad = np.zeros((B, VP), dtype=np.float16)
    bow_pad[:, :V] = BOW.astype(np.float16)

    in_maps = []
    for c in range(NCORES):
        lo, hi = c * VPAD, (c + 1) * VPAD
        bc = bow_pad[:, lo:hi]
        bow_pack = np.ascontiguousarray(
            bc.reshape(4, 128, NBLK, 512).transpose(2, 1, 0, 3).reshape(NBLK, 128, 2048)
        )
        sc = s_pad[lo:hi]
        s_pack = np.ascontiguousarray(
            sc.reshape(NBLK, 4, 128, K).transpose(0, 2, 1, 3).reshape(NBLK, 128, 200)
        )
        ec = e_pad[lo:hi]
        e_pack = np.ascontiguousarray(
            ec.reshape(NBLK, 4, 128, K).transpose(0, 2, 1, 3).reshape(NBLK, 128, 200)
        )
        in_maps.append(
            {
                "bow": bow_pack,
                "sT": np.ascontiguousarray(sT_pad[:, lo:hi]),
                "s": s_pack,
                "e": e_pack,
                "aT1": aT1,
                "a2": a2_pack,
            }
        )

    nc = _get_nc()
    res = run_bass_kernel_spmd(nc, in_maps, list(range(NCORES)))
    _last_results = res

    shards = []
    for c in range(NCORES):
        o = np.asarray(res.results[c]["out"])  # [NBLK, 128, 200]
        o = o.reshape(NBLK, 128, 4, K).transpose(0, 2, 1, 3).reshape(VPAD, K)
        shards.append(o)
    # cores hold contiguous 12800-row slices of the padded [102400, K] vocab;
    # the pad is entirely at the global tail.
    return np.concatenate(shards, axis=0)[:V].astype(np.float32)


# revision 21
# speedup vs baseline: 1.1623x; 1.1623x over previous
"""Trainium2 Bass kernel for the MixEHR SCVB0_un step (nn_MixEHR_5428838662489).

Math (see reference):
    a     = alpha + exp_m[batch_indices]                  [B, K]
    denom = beta.sum(0) + exp_n.sum(0)                    [K]
    b     = (beta + exp_n) / denom                        [V, K]
    Z     = a @ b.T                                       [B, V]
    W     = BOW / (Z + 1e-6)                              [B, V]
    out   = (1-rho) * exp_n + rho*scale * b * (W.T @ a)   [V, K]

Device strategy: shard the vocabulary V across the 8 cores (no collectives
needed — every core holds all B=512 documents and produces a complete
[V/8, K] output shard).  All matmul factors are prefolded on the host:

    s   = beta + exp_n                 (f32, natural + f16 transposed)
    aT1 = (a / denom).T                (f16)   Z = aT1.T @ sT
    a2  = a * (rho*scale/denom)        (f16)   P = W.T @ a2
    e   = (1-rho) * exp_n              (f32)
    out = e + s * P

Per core the vocab shard is padded 12500 -> 12800 = 25 blocks of 512.
Per block: 4 matmuls build Z [128d, 4*512v] in PSUM; reciprocal via
ACT exp(-ln(Z+eps)) (or DVE reciprocal_approx_fast on some units);
W = BOW * R on DVE (f16); 16 matmuls accumulate P [128v, 4*50k];
DVE combines out = e + s*P.
"""

import numpy as np

import bass_rust as _bass_rust
import concourse.bass as bass
import concourse.mybir as mybir
import concourse.tile as tile
from concourse import bacc
from concourse.bass_utils import run_bass_kernel_spmd
from concourse.hw_specs import get_activation_tables

B = 512          # documents (batch)
V = 100000       # vocabulary
K = 50           # topics
NCORES = 8
VCORE = 12500    # true vocab per core
VPAD = 12800     # padded vocab per core
NBLK = 25        # blocks of 512 vocab per core
MINI = 1e-6

F16 = mybir.dt.float16
F32 = mybir.dt.float32

# Blocks with (blk % 5) in this set compute 1/Z with DVE reciprocal_approx_fast
# instead of ACT exp(-ln(Z+eps)) — balances DVE vs ACT engine time.
DVE_BLOCK_SET = (0, 3)
# On ACT-path blocks, run the W = BOW*R multiply on GpSimd instead of DVE.
GPS_MUL = True

_CACHE = {}
_last_results = None  # test harness reads timing info from here


class _Bacc(bacc.Bacc):
    """Bacc with the activation-table chooser pinned to the one set that has
    BOTH Exp and Ln (`natural_log_exp_and_others`), so alternating Ln/Exp ops
    don't thrash ACT_TABLE_LOADs (~1 µs each, 2 per block otherwise)."""

    def insert_act_table_loads(self):
        has_activation = any(
            isinstance(i, mybir.InstActivation)
            for b in self.main_func.blocks
            for i in b.instructions
        )
        if not has_activation:
            return
        exp_fn = mybir.ActivationFunctionType.Exp
        ln_fn = mybir.ActivationFunctionType.Ln
        tables = []
        for name, fns in get_activation_tables(self.m.arch).items():
            if name != "natural_log_exp_and_others":
                fns = {f for f in fns if f not in (exp_fn, ln_fn)}
            tables.append((name, fns))
        _bass_rust.insert_act_table_loads(self, tables)


def _build_nc():
    nc = _Bacc("TRN2", target_bir_lowering=False)
    # Register the eps bias constant as an untracked const AP (written in the
    # preamble, before the TileContext) so activations using it need no
    # cross-engine sync wait — the ACT instruction has one wait slot.
    _eps = nc.alloc_sbuf_tensor("const-eps", [128, 1], F32)
    nc.gpsimd.memset(_eps.ap(), float(MINI))
    nc.const_aps.aps[(F32, float(MINI))] = _eps.ap()
    nc.all_engine_barrier()
    bow = nc.declare_dram_parameter("bow", [NBLK, 128, 2048], F16, isOutput=False)
    sT = nc.declare_dram_parameter("sT", [K, VPAD], F16, isOutput=False)
    s_p = nc.declare_dram_parameter("s", [NBLK, 128, 200], F32, isOutput=False)
    e_p = nc.declare_dram_parameter("e", [NBLK, 128, 200], F32, isOutput=False)
    aT1 = nc.declare_dram_parameter("aT1", [K, B], F16, isOutput=False)
    a2 = nc.declare_dram_parameter("a2", [128, 200], F16, isOutput=False)
    out = nc.declare_dram_parameter("out", [NBLK, 128, 200], F32, isOutput=True)

    with tile.TileContext(nc) as tc:
        with (
            tc.tile_pool(name="consts", bufs=1) as consts,
            tc.tile_pool(name="bowp", bufs=3) as bowp,
            tc.tile_pool(name="zp", bufs=3, space="PSUM") as zpool,
            tc.tile_pool(name="pp", bufs=2, space="PSUM") as ppool,
            tc.tile_pool(name="tp", bufs=4) as tpool,
            tc.tile_pool(name="rp", bufs=2) as rpool,
            tc.tile_pool(name="wp", bufs=2) as wpool,
            tc.tile_pool(name="sep", bufs=4) as sepool,
            tc.tile_pool(name="op", bufs=3) as opool,
        ):
            sT_t = consts.tile([K, VPAD], F16)
            nc.sync.dma_start(out=sT_t, in_=sT[:])
            aT1_t = consts.tile([K, B], F16)
            nc.sync.dma_start(out=aT1_t, in_=aT1[:])
            a2_t = consts.tile([128, 200], F16)
            nc.sync.dma_start(out=a2_t, in_=a2[:])


            for blk in range(NBLK):
                bow_t = bowp.tile([128, 2048], F16, tag="bow")
                nc.sync.dma_start(out=bow_t, in_=bow[blk])
                s_t = sepool.tile([128, 200], F32, tag="s")
                nc.sync.dma_start(out=s_t, in_=s_p[blk])
                e_t = sepool.tile([128, 200], F32, tag="e")
                nc.sync.dma_start(out=e_t, in_=e_p[blk])

                # ---- Z = aT1.T @ sT per doc-chunk: [128d, 512v] slices, two
                # chunks per [128, 1024] psum tile (2 banks) ----
                zp_a = zpool.tile([128, 1024], F32, tag="z")
                zp_b = zpool.tile([128, 1024], F32, tag="z")
                zp_h = [zp_a, zp_b]
                for c in range(4):
                    nc.tensor.matmul(
                        zp_h[c // 2][:, (c % 2) * 512 : (c % 2) * 512 + 512],
                        lhsT=aT1_t[:, c * 128 : (c + 1) * 128],
                        rhs=sT_t[:, blk * 512 : (blk + 1) * 512],
                        start=True,
                        stop=True,
                    )

                # ---- R = 1/(Z+eps), then W = BOW * R (f16) ----
                if (blk % 5) in DVE_BLOCK_SET:
                    rf_t = tpool.tile([128, 2048], F32, tag="t")
                    for h in range(2):
                        nc.vector.reciprocal_approx_fast(
                            out=rf_t[:, h * 1024 : (h + 1) * 1024], in_=zp_h[h]
                        )
                    w_t = wpool.tile([128, 2048], F16, tag="w")
                    nc.vector.tensor_mul(w_t, bow_t, rf_t)
                else:
                    t_t = tpool.tile([128, 2048], F32, tag="t")
                    for h in range(2):
                        nc.scalar.activation(
                            t_t[:, h * 1024 : (h + 1) * 1024],
                            zp_h[h],
                            mybir.ActivationFunctionType.Ln,
                            bias=float(MINI),
                            scale=1.0,
                        )
                    r_t = rpool.tile([128, 2048], F16, tag="r")
                    nc.scalar.activation(
                        r_t,
                        t_t,
                        mybir.ActivationFunctionType.Exp,
                        bias=0.0,
                        scale=-1.0,
                    )
                    w_t = wpool.tile([128, 2048], F16, tag="w")
                    if GPS_MUL:
                        nc.gpsimd.tensor_mul(w_t, bow_t, r_t)
                    else:
                        nc.vector.tensor_mul(w_t, bow_t, r_t)

                # ---- P[v,k] = sum_d W[d,v] * a2[d,k], accumulated over 4 d-chunks
                p_t = ppool.tile([128, 200], F32, tag="p")
                for vs in range(4):
                    for c in range(4):
                        nc.tensor.matmul(
                            p_t[:, vs * 50 : vs * 50 + 50],
                            lhsT=w_t[:, c * 512 + vs * 128 : c * 512 + vs * 128 + 128],
                            rhs=a2_t[:, c * 50 : (c + 1) * 50],
                            start=(c == 0),
                            stop=(c == 3),
                        )

                # ---- out = e + s * P ----
                u_t = opool.tile([128, 200], F32, tag="u")
                nc.vector.tensor_mul(u_t, s_t, p_t)
                o_t = opool.tile([128, 200], F32, tag="o")
                nc.vector.tensor_add(o_t, u_t, e_t)
                nc.sync.dma_start(out=out[blk], in_=o_t)

    nc.compile()
    return nc


def _get_nc():
    if "nc" not in _CACHE:
        _CACHE["nc"] = _build_nc()
    return _CACHE["nc"]


def kernel(
    batch_BOW,
    alpha,
    beta,
    exp_m,
    exp_n,
    batch_indices,
    iter_n,
    batch_C,
    C_m,
):
    global _last_results
    BOW = np.asarray(batch_BOW, dtype=np.float32)
    alpha = np.asarray(alpha, dtype=np.float32)
    beta = np.asarray(beta, dtype=np.float32)
    exp_m = np.asarray(exp_m, dtype=np.float32)
    exp_n = np.asarray(exp_n, dtype=np.float32)
    bidx = np.asarray(batch_indices)

    rho = 1.0 / float(int(iter_n) + 5) ** 0.9
    scale = float(C_m) / float(batch_C)

    # ---- host prefolding (O(V*K) / O(B*K) prep) ----
    denom = (
        beta.sum(axis=0, dtype=np.float64) + exp_n.sum(axis=0, dtype=np.float64)
    ).astype(np.float32)
    a = alpha[None, :] + exp_m[bidx]                       # [B, K]
    aT1 = np.ascontiguousarray((a / denom[None, :]).T.astype(np.float16))
    a2 = (a * (rho * scale / denom)[None, :]).astype(np.float16)
    a2_pack = np.ascontiguousarray(
        a2.reshape(4, 128, K).transpose(1, 0, 2).reshape(128, 200)
    )
    s = beta + exp_n                                       # [V, K] f32

    VP = VPAD * NCORES
    sT_pad = np.ones((K, VP), dtype=np.float16)
    sT_pad[:, :V] = s.T.astype(np.float16)
    s_pad = np.zeros((VP, K), dtype=np.float32)
    s_pad[:V] = s
    e_pad = np.zeros((VP, K), dtype=np.float32)
    e_pad[:V] = (1.0 - rho) * exp_n
    bow_pad = np.zeros((B, VP), dtype=np.float16)
    bow_pad[:, :V] = BOW.astype(np.float16)

    in_maps = []
    for c in range(NCORES):
        lo, hi = c * VPAD, (c + 1) * VPAD
        bc = bow_pad[:, lo:hi]
        bow_pack = np.ascontiguousarray(
            bc.reshape(4, 128, NBLK, 512).transpose(2, 1, 0, 3).reshape(NBLK, 128, 2048)
        )
        sc = s_pad[lo:hi]
        s_pack = np.ascontiguousarray(
            sc.reshape(NBLK, 4, 128, K).transpose(0, 2, 1, 3).reshape(NBLK, 128, 200)
        )
        ec = e_pad[lo:hi]
        e_pack = np.ascontiguousarray(
            ec.reshape(NBLK, 4, 128, K).transpose(0, 2, 1, 3).reshape(NBLK, 128, 200)
        )
        in_maps.append(
            {
                "bow": bow_pack,
                "sT": np.ascontiguousarray(sT_pad[:, lo:hi]),
                "s": s_pack,
                "e": e_pack,
                "aT1": aT1,
                "a2": a2_pack,
            }
        )

    nc = _get_nc()
    res = run_bass_kernel_spmd(nc, in_maps, list(range(NCORES)))
    _last_results = res

    shards = []
    for c in range(NCORES):
        o = np.asarray(res.results[c]["out"])  # [NBLK, 128, 200]
        o = o.reshape(NBLK, 128, 4, K).transpose(0, 2, 1, 3).reshape(VPAD, K)
        shards.append(o)
    # cores hold contiguous 12800-row slices of the padded [102400, K] vocab;
    # the pad is entirely at the global tail.
    return np.concatenate(shards, axis=0)[:V].astype(np.float32)
